# revision 1
# baseline (speedup 1.0000x reference)
"""MultiHeadAttention TRN2 Bass kernel (8 NeuronCores).

Sharding: core c = (batch b = c//2, query-half = c%2). Each core computes
K/V for its full batch (2048 keys) and attention + output projection + LN
for its 1024 query rows. No collectives; host gathers per-core outputs.

Device math (all matmuls in float32r = full-rate fp32, rel err ~2e-4):
  QhT[hd, q]  = wq[d, hd].T @ qT[d, q]          (per 4-head phase)
  KhT[hd, m]  = wk[d, hd].T @ qT[d, m]
  Vaug[m, 65] = qT[d, m].T @ wv[d, 65-packed]   (+ ones column)
  S^T[m, q]   = KhT[dk, m].T @ QhT[dk, q]       (K=64, head pairs packed
                                                 via tile_position rows)
  E = exp(S^T / 32)                              (ACT, PSUM->SBUF fp32r)
  OT[65, q]   = sum_m Vaug[m,65].T @ E[m, q]    (row 64 = softmax denom)
  CT[c, q]    = OT[0:64] * (1/denom)            (K=1 ones-matmul bcast +
                                                 reciprocal_approx_fast)
  Y[q, o]     = CT[c, q].T @ pwT[c, o] + (q_res + proj_b)   then LayerNorm
"""
import numpy as np

import concourse.bass as bass
import concourse.mybir as mybir
import concourse.tile as tile
from concourse import bacc
from concourse.bass_utils import run_bass_kernel_spmd

F32 = mybir.dt.float32
F32R = mybir.dt.float32r
AF = mybir.ActivationFunctionType
ALU = mybir.AluOpType
AX = mybir.AxisListType

B, L, D = 4, 2048, 1024
H, DK = 16, 64
HALF = 1024            # query rows per core
TEMPER = 32.0          # sqrt(d_model)
PHASES = 4
HP = H // PHASES       # 4 heads per phase
PAIRS = HP // 2        # 2 head-pairs per phase
MT = L // 128          # 16 m-tiles
LN_EPS = 1e-3

_CACHE = {}


def build(iters=1):
    nc = bacc.Bacc(None, target_bir_lowering=False)
    qt_d = nc.dram_tensor("qt", [D, L], F32R, kind="ExternalInput")
    qres_d = nc.dram_tensor("qres", [HALF, D], F32, kind="ExternalInput")
    wq_d = nc.dram_tensor("wq", [D, H * DK], F32R, kind="ExternalInput")
    wk_d = nc.dram_tensor("wk", [D, H * DK], F32R, kind="ExternalInput")
    wv_d = nc.dram_tensor("wv", [D, H * 65], F32R, kind="ExternalInput")
    pw_d = nc.dram_tensor("pw", [D, D], F32R, kind="ExternalInput")
    lna_d = nc.dram_tensor("lna", [1, D], F32, kind="ExternalInput")
    lnb_d = nc.dram_tensor("lnb", [1, D], F32, kind="ExternalInput")
    ones_d = nc.dram_tensor("ones64", [1, 64], F32R, kind="ExternalInput")
    out_d = nc.dram_tensor("out", [HALF, D], F32, kind="ExternalOutput")

    with tile.TileContext(nc) as tc:
        with (
            tc.tile_pool(name="p1", bufs=1) as p1,
            tc.tile_pool(name="p2", bufs=2) as p2,
            tc.tile_pool(name="psA", bufs=4, space="PSUM") as psA,
            tc.tile_pool(name="psS", bufs=2, space="PSUM") as psS,
        ):
            # ---- one-time constants ----
            ones_t = p1.tile([128, 64], F32R, name="ones_t")
            nc.sync.dma_start(ones_t[64:65, :], ones_d[:])
            ones_sb = p1.tile([128, 16], F32R, name="ones_sb")
            nc.sync.dma_start(ones_sb[:], ones_d[:, 0:16].to_broadcast([128, 16]))
            lna_t = p1.tile([128, D], F32, name="lna_t")
            nc.sync.dma_start(lna_t[:], lna_d[:].to_broadcast([128, D]))
            lnb_t = p1.tile([128, D], F32, name="lnb_t")
            nc.sync.dma_start(lnb_t[:], lnb_d[:].to_broadcast([128, D]))

            ct_t = p1.tile([128, H // 2, HALF], F32R, name="ct_t")

            for it in range(iters):
                for p in range(PHASES):
                    c0 = p * HP * DK
                    wq_t = p1.tile([128, 8, HP * DK], F32R, name=f"it{it}_wq_{p}", tag="wq")
                    nc.sync.dma_start(
                        wq_t[:],
                        wq_d[:, c0:c0 + HP * DK].rearrange("(dj pp) f -> pp dj f", pp=128),
                    )
                    wk_t = p1.tile([128, 8, HP * DK], F32R, name=f"it{it}_wk_{p}", tag="wk")
                    nc.sync.dma_start(
                        wk_t[:],
                        wk_d[:, c0:c0 + HP * DK].rearrange("(dj pp) f -> pp dj f", pp=128),
                    )
                    v0 = p * HP * 65
                    wv_t = p1.tile([128, 8, HP * 65], F32R, name=f"it{it}_wv_{p}", tag="wv")
                    nc.sync.dma_start(
                        wv_t[:],
                        wv_d[:, v0:v0 + HP * 65].rearrange("(dj pp) f -> pp dj f", pp=128),
                    )

                    qht_t = p2.tile([128, PAIRS, HALF], F32R, name=f"it{it}_qht_{p}", tag="qht")
                    kht_t = p2.tile([128, PAIRS, L], F32R, name=f"it{it}_kht_{p}", tag="kht")
                    vaug_t = p2.tile([128, MT, HP * 65], F32R, name=f"it{it}_vaug_{p}", tag="vaug")

                    # ---- QKV projections, streaming qT in 512-col blocks ----
                    for mc in range(L // 512):
                        qt_t = p2.tile([128, 8, 512], F32R, name=f"it{it}_qt_{p}_{mc}", tag="qt")
                        nc.sync.dma_start(
                            qt_t[:],
                            qt_d[:, mc * 512:(mc + 1) * 512].rearrange(
                                "(dj pp) m -> pp dj m", pp=128
                            ),
                        )
                        # K (and Q for the first half of columns): 2+2 psum groups
                        for mt in range(PAIRS):
                            kps = psA.tile([128, 512], F32, name=f"it{it}_kps_{p}_{mc}_{mt}",
                                           tag="acc")
                            for dj in range(8):
                                nc.tensor.matmul(
                                    kps[:],
                                    wk_t[:, dj, mt * 128:(mt + 1) * 128],
                                    qt_t[:, dj, :],
                                    start=(dj == 0), stop=(dj == 7),
                                )
                            nc.vector.tensor_copy(
                                kht_t[:, mt, mc * 512:(mc + 1) * 512], kps[:]
                            )
                        if mc < HALF // 512:
                            for mt in range(PAIRS):
                                qps = psA.tile([128, 512], F32, name=f"it{it}_qps_{p}_{mc}_{mt}",
                                               tag="acc")
                                for dj in range(8):
                                    nc.tensor.matmul(
                                        qps[:],
                                        wq_t[:, dj, mt * 128:(mt + 1) * 128],
                                        qt_t[:, dj, :],
                                        start=(dj == 0), stop=(dj == 7),
                                    )
                                nc.vector.tensor_copy(
                                    qht_t[:, mt, mc * 512:(mc + 1) * 512], qps[:]
                                )
                        # V: 4 m-subtiles of 128, N = HP*65 = 260
                        for ms in range(4):
                            mi = mc * 4 + ms
                            vps = psA.tile([128, HP * 65], F32, name=f"it{it}_vps_{p}_{mi}",
                                           tag="acc")
                            for dj in range(8):
                                nc.tensor.matmul(
                                    vps[:],
                                    qt_t[:, dj, ms * 128:(ms + 1) * 128],
                                    wv_t[:, dj, :],
                                    start=(dj == 0), stop=(dj == 7),
                                )
                            nc.vector.tensor_copy(vaug_t[:, mi, :], vps[:])
                    # ones columns of V_aug
                    for hl in range(HP):
                        nc.vector.tensor_copy(
                            vaug_t[:, :, hl * 65 + 64], ones_sb[:, 0:MT]
                        )

                    # ---- attention ----
                    for a in range(PAIRS):
                        cj = p * PAIRS + a
                        for qc in range(HALF // 512):
                            qs = slice(qc * 512, (qc + 1) * 512)
                            ot = {}
                            for par in range(2):
                                ot[par] = psA.tile([65, 512], F32,
                                                   name=f"it{it}_ot_{p}_{a}_{qc}_{par}",
                                                   tag="acc")
                            for mi in range(MT):
                                ms_ = slice(mi * 128, (mi + 1) * 128)
                                sp = psS.tile([128, 1024], F32,
                                              name=f"it{it}_s_{p}_{a}_{qc}_{mi}",
                                              tag="score")
                                for par in range(2):
                                    nc.tensor.matmul(
                                        sp[:, 512 * par:512 * (par + 1)],
                                        kht_t[64 * par:64 * (par + 1), a, ms_],
                                        qht_t[64 * par:64 * (par + 1), a, qs],
                                        start=True, stop=True,
                                        tile_position=(64 * par, 0),
                                    )
                                ex = p2.tile([128, 1024], F32R,
                                             name=f"it{it}_e_{p}_{a}_{qc}_{mi}",
                                             tag="exp")
                                nc.scalar.activation(ex[:], sp[:], AF.Exp,
                                                     scale=1.0 / TEMPER)
                                for par in range(2):
                                    hl = 2 * a + par
                                    nc.tensor.matmul(
                                        ot[par][:],
                                        vaug_t[:, mi, hl * 65:(hl + 1) * 65],
                                        ex[:, 512 * par:512 * (par + 1)],
                                        start=(mi == 0), stop=(mi == MT - 1),
                                    )
                            for par in range(2):
                                den = p1.tile([128, 512], F32R,
                                              name=f"it{it}_den_{p}_{a}_{qc}_{par}", tag="den")
                                nc.vector.tensor_copy(den[64:65, :], ot[par][64:65, :])
                                bc = psA.tile([64, 512], F32,
                                              name=f"it{it}_bc_{p}_{a}_{qc}_{par}", tag="acc")
                                nc.tensor.matmul(bc[:], ones_t[64:65, :],
                                                 den[64:65, :], start=True, stop=True)
                                rec = p1.tile([64, 512], F32,
                                              name=f"it{it}_rec_{p}_{a}_{qc}_{par}", tag="rec")
                                nc.vector.reciprocal_approx_fast(rec[:], bc[:])
                                if par == 0:
                                    nc.vector.tensor_mul(
                                        ct_t[0:64, cj, qs], ot[par][0:64, :], rec[:]
                                    )
                                else:
                                    stg = p1.tile([64, 512], F32R,
                                                  name=f"it{it}_stg_{p}_{a}_{qc}", tag="stg")
                                    nc.vector.tensor_mul(stg[:], ot[par][0:64, :], rec[:])
                                    nc.sync.dma_start(ct_t[64:128, cj, qs], stg[:])

                # ---- output projection + residual + layernorm ----
                pw_t = {}
                for oc in range(2):
                    pw_t[oc] = p2.tile([128, 8, 512], F32R,
                                       name=f"it{it}_pwt_{oc}", tag="kht")
                    nc.sync.dma_start(
                        pw_t[oc][:],
                        pw_d[:, oc * 512:(oc + 1) * 512].rearrange(
                            "(dj pp) f -> pp dj f", pp=128
                        ),
                    )
                for qtb in range(4):
                    yts = {}
                    for qt in range(2):
                        yts[qt] = p2.tile([128, D], F32,
                                          name=f"it{it}_yt_{qtb}_{qt}", tag="y")
                        qti0 = qtb * 2 + qt
                        nc.sync.dma_start(
                            yts[qt][:], qres_d[qti0 * 128:(qti0 + 1) * 128, :])
                    for oc in range(2):
                        ypss = {}
                        for cjj in range(H // 2):
                            for qt in range(2):
                                qti = qtb * 2 + qt
                                if cjj == 0:
                                    ypss[qt] = psA.tile(
                                        [128, 512], F32,
                                        name=f"it{it}_y_{qtb}_{oc}_{qt}", tag="acc")
                                nc.tensor.matmul(
                                    ypss[qt][:],
                                    ct_t[:, cjj, qti * 128:(qti + 1) * 128],
                                    pw_t[oc][:, cjj, :],
                                    start=(cjj == 0), stop=(cjj == H // 2 - 1),
                                )
                        for qt in range(2):
                            nc.vector.tensor_add(
                                yts[qt][:, oc * 512:(oc + 1) * 512],
                                yts[qt][:, oc * 512:(oc + 1) * 512],
                                ypss[qt][:],
                            )
                    for qt in range(2):
                        qti = qtb * 2 + qt
                        y_t = yts[qt]
                        # layernorm: mu, sigma (ddof=1), (y-mu)/(sigma+eps)*a+b
                        s = p1.tile([128, 1], F32, name=f"it{it}_s_{qti}", tag="ln_s")
                        nc.vector.reduce_sum(s[:], y_t[:], axis=AX.X)
                        negmean = p1.tile([128, 1], F32, name=f"it{it}_nm_{qti}", tag="ln_nm")
                        nc.vector.tensor_scalar_mul(negmean[:], s[:], -1.0 / D)
                        mean = p1.tile([128, 1], F32, name=f"it{it}_m_{qti}", tag="ln_m")
                        nc.vector.tensor_scalar_mul(mean[:], s[:], 1.0 / D)
                        ss = p1.tile([128, 1], F32, name=f"it{it}_ss_{qti}", tag="ln_ss")
                        ss2 = p1.tile([128, 1], F32, name=f"it{it}_ss2_{qti}", tag="ln_ss2")
                        for oc in range(2):
                            sq = psS.tile([128, 512], F32, name=f"it{it}_sq_{qti}_{oc}",
                                          tag="score")
                            nc.scalar.activation(
                                sq[:], y_t[:, oc * 512:(oc + 1) * 512], AF.Square,
                                bias=negmean[:],
                                accum_out=(ss[:] if oc == 0 else ss2[:]),
                            )
                        nc.vector.tensor_add(ss[:], ss[:], ss2[:])
                        sigma = p1.tile([128, 1], F32, name=f"it{it}_sg_{qti}", tag="ln_sg")
                        nc.scalar.activation(sigma[:], ss[:], AF.Sqrt,
                                             scale=1.0 / (D - 1))
                        var = p1.tile([128, 1], F32, name=f"it{it}_var_{qti}", tag="ln_var")
                        nc.vector.tensor_scalar_mul(var[:], ss[:], 1.0 / (D - 1))
                        rs = p1.tile([128, 1], F32, name=f"it{it}_rs_{qti}", tag="ln_rs")
                        nc.vector.reciprocal(rs[:], sigma[:])
                        t1 = p1.tile([128, 1], F32, name=f"it{it}_t1_{qti}", tag="ln_t1")
                        nc.vector.tensor_mul(t1[:], var[:], rs[:])
                        nc.vector.tensor_add(t1[:], t1[:], sigma[:])
                        dd = p1.tile([128, 1], F32, name=f"it{it}_dd_{qti}", tag="ln_dd")
                        nc.vector.tensor_scalar(dd[:], t1[:], 0.5, LN_EPS,
                                                ALU.mult, ALU.add)
                        rec2 = p1.tile([128, 1], F32, name=f"it{it}_rc_{qti}", tag="ln_rc")
                        nc.vector.reciprocal(rec2[:], dd[:])
                        o_t = p2.tile([128, D], F32, name=f"it{it}_o_{qti}", tag="o")
                        nc.vector.tensor_scalar(o_t[:], y_t[:], mean[:], rec2[:],
                                                ALU.subtract, ALU.mult)
                        nc.vector.tensor_mul(o_t[:], o_t[:], lna_t[:])
                        nc.vector.tensor_add(o_t[:], o_t[:], lnb_t[:])
                        nc.sync.dma_start(out_d[qti * 128:(qti + 1) * 128, :], o_t[:])

    nc.compile()
    return nc


def _get_nc():
    if "nc" not in _CACHE:
        _CACHE["nc"] = build()
    return _CACHE["nc"]


def kernel(q, w_qs, w_ks, w_vs, proj_w, proj_b, ln_a, ln_b, **kw):
    q = np.asarray(q, dtype=np.float32)
    w_qs = np.asarray(w_qs, dtype=np.float32)
    w_ks = np.asarray(w_ks, dtype=np.float32)
    w_vs = np.asarray(w_vs, dtype=np.float32)
    proj_w = np.asarray(proj_w, dtype=np.float32)
    proj_b = np.asarray(proj_b, dtype=np.float32)
    ln_a = np.asarray(ln_a, dtype=np.float32)
    ln_b = np.asarray(ln_b, dtype=np.float32)

    wq_all = np.ascontiguousarray(w_qs.transpose(1, 0, 2).reshape(D, H * DK))
    wk_all = np.ascontiguousarray(w_ks.transpose(1, 0, 2).reshape(D, H * DK))
    wv_aug = np.zeros((D, H, 65), dtype=np.float32)
    wv_aug[:, :, :64] = w_vs.transpose(1, 0, 2)
    wv_aug = np.ascontiguousarray(wv_aug.reshape(D, H * 65))
    pwT = np.ascontiguousarray(proj_w.T)
    ones64 = np.ones((1, 64), dtype=np.float32)
    lna = np.ascontiguousarray(ln_a[None, :])
    lnb = np.ascontiguousarray(ln_b[None, :])

    in_maps = []
    for c in range(8):
        b, half = c // 2, c % 2
        qbT = q[b].T  # [D, L]
        qt_c = np.ascontiguousarray(
            np.concatenate(
                [qbT[:, half * HALF:(half + 1) * HALF],
                 qbT[:, (1 - half) * HALF:(2 - half) * HALF]],
                axis=1,
            )
        )
        qres_c = np.ascontiguousarray(
            q[b, half * HALF:(half + 1) * HALF, :] + proj_b[None, :]
        )
        in_maps.append({
            "qt": qt_c, "qres": qres_c,
            "wq": wq_all, "wk": wk_all, "wv": wv_aug, "pw": pwT,
            "lna": lna, "lnb": lnb, "ones64": ones64,
        })

    nc = _get_nc()
    res = run_bass_kernel_spmd(nc, in_maps, core_ids=list(range(8))).results

    out = np.empty((B, L, D), dtype=np.float32)
    for c in range(8):
        b, half = c // 2, c % 2
        out[b, half * HALF:(half + 1) * HALF, :] = res[c]["out"]
    return out



# revision 9
# speedup vs baseline: 1.6446x; 1.6446x over previous
"""MultiHeadAttention TRN2 Bass kernel (8 NeuronCores), fp8 DoubleRow version.

Sharding: core c = (batch b = c//2, query-half = c%2). Each core computes
K/V for its full batch (2048 keys) and attention + output projection + LN
for its 1024 query rows. No collectives; host gathers per-core outputs.

Device math (fp8e4m3 matmuls in DoubleRow perf mode = 2 k-tiles of 128
contracted per instruction at 0.5 cycles per output column):
  Q/K proj: out [hd(2 heads x 64dk), m]  = w8[d,2,hd].T @ qt8[d,2,m]
  V proj:   out [m, 2 heads x 64]        = qt8[d,2,m].T @ wv8[d,2,hv]
  kht2/qht2: DMA re-layout to [32, 2(dk-half), m] per head so S can run
  DoubleRow with dk = 2x32.
  S[m, q]   = kht2[32,2,mtile].T @ qht2[32,2,q]      (per head, per m-tile)
  E = exp(S/32): split between ACT (Exp) and a custom DVE op
      (0.5*(1+x/8)^2+0.5)^8 ~ e^x with <1% typical error.
  O[q, 64]  = sum_p E[m,2,q].T @ V[m,2,64]           (E is the stationary)
  den[q]    = sum_p E[m,2,q].T @ ones[m,2,1]
  CT[q, (h,64)] = O * (1/den)    (per-partition scalar broadcast)
  ctT = DMA-transpose(CT bf16) -> fp8;  Y = ctT.T @ pw8 + residual; LayerNorm
"""
import numpy as np
import ml_dtypes

import concourse.bass as bass
import concourse.mybir as mybir
import concourse.tile as tile
from concourse import bacc
from concourse.bass_utils import run_bass_kernel_spmd

# ---------------- custom DVE exp op ----------------
import concourse.dve_ops as dve_ops
from concourse.dve_spec import Spec, Src0, C0, C1, C2, sq


def _exp_ref(in0, in1, c0, c1, c2):
    x = in0.astype(np.float32)
    t = ((x * c0 + c1) ** 2 * c2 + c2).astype(np.float32)
    t = (t * t).astype(np.float32)
    t = (t * t).astype(np.float32)
    return (t * t).astype(np.float32)


def _get_exp_op():
    name = "EXP_APPROX_ANT"
    for op in dve_ops.OPS:
        if op.name == name:
            return op
    body = sq(sq(sq(sq(Src0 * C0 + C1) * C2 + C2)))
    op = dve_ops.DveOp(
        name,
        Spec(body=body, reference=_exp_ref),
        subdim=False,
        uops_sha={"v3": "7cc04e385f99d2ac", "v4": "4c6dc6b0499997cd"},
    )
    row = max(dve_ops._SUB_OPCODE_FOR_NAME.values()) + 1
    assert row < 0x20
    dve_ops.OPS.append(op)
    dve_ops.CUSTOM_DVE_SPECS[name] = op.spec
    dve_ops._SUB_OPCODE_FOR_NAME[name] = row
    return op


EXP_OP = _get_exp_op()

F32 = mybir.dt.float32
BF16 = mybir.dt.bfloat16
F8 = mybir.dt.float8e4
AF = mybir.ActivationFunctionType
ALU = mybir.AluOpType
AX = mybir.AxisListType
PM = mybir.MatmulPerfMode

B, L, D = 4, 2048, 1024
H, DK = 16, 64
HALF = 1024
TEMPER = 32.0
LN_EPS = 1e-3
EA = 576  # columns of each 1024-wide exp tile handled by ACT (rest on DVE)
NP8 = ml_dtypes.float8_e4m3

_CACHE = {}


def build(iters=1):
    nc = bacc.Bacc(None, target_bir_lowering=False)
    qt8_d = nc.dram_tensor("qt8", [128, 8, L], F8, kind="ExternalInput")
    wq8_d = nc.dram_tensor("wq8", [128, 8, H * DK], F8, kind="ExternalInput")
    wk8_d = nc.dram_tensor("wk8", [128, 8, H * DK], F8, kind="ExternalInput")
    wv8_d = nc.dram_tensor("wv8", [128, 8, H * DK], F8, kind="ExternalInput")
    pw8_d = nc.dram_tensor("pw8", [128, 8, D], F8, kind="ExternalInput")
    ones_d = nc.dram_tensor("ones8", [1, 2], F8, kind="ExternalInput")
    qres_d = nc.dram_tensor("qres", [HALF, D], F32, kind="ExternalInput")
    lna_d = nc.dram_tensor("lna", [1, D], F32, kind="ExternalInput")
    lnb_d = nc.dram_tensor("lnb", [1, D], F32, kind="ExternalInput")
    out_d = nc.dram_tensor("out", [HALF, D], F32, kind="ExternalOutput")

    with tile.TileContext(nc) as tc:
        with (
            tc.tile_pool(name="c1", bufs=1) as c1,
            tc.tile_pool(name="st", bufs=2) as st,
            tc.tile_pool(name="eh", bufs=2) as ehp,
            tc.tile_pool(name="sm", bufs=2) as smp,
            tc.tile_pool(name="psQ", bufs=2, space="PSUM") as psQ,
            tc.tile_pool(name="psD", bufs=1, space="PSUM") as psD,
            tc.tile_pool(name="psS", bufs=2, space="PSUM") as psS,
            tc.tile_pool(name="psO", bufs=1, space="PSUM") as psO,
        ):
            # persistent SBUF
            qt8 = c1.tile([128, 8, L], F8, name="qt8_t")
            wq8 = c1.tile([128, 8, H * DK], F8, name="wq8_t")
            wk8 = c1.tile([128, 8, H * DK], F8, name="wk8_t")
            wv8 = c1.tile([128, 8, H * DK], F8, name="wv8_t")
            pw8 = c1.tile([128, 8, D], F8, name="pw8_t")
            ones8 = c1.tile([128, 2, 1], F8, name="ones8_t")
            kht2 = c1.tile([128, 6, 2, L], F8, name="kht2_t")
            qht2 = c1.tile([128, 6, 2, HALF], F8, name="qht2_t")
            vaug = c1.tile([128, 16, H * DK], F8, name="vaug_t")
            ct = c1.tile([128, 8, D], BF16, name="ct_t")
            ctT8 = c1.tile([128, 8, D], F8, name="ctT8_t")
            lna_t = c1.tile([128, D], F32, name="lna_t")
            lnb_t = c1.tile([128, D], F32, name="lnb_t")

            nc.sync.dma_start(qt8[:], qt8_d[:])
            nc.sync.dma_start(wq8[:], wq8_d[:])
            nc.sync.dma_start(wk8[:], wk8_d[:])
            nc.sync.dma_start(wv8[:], wv8_d[:])
            nc.sync.dma_start(pw8[:], pw8_d[:])
            nc.sync.dma_start(ones8[:, :, 0], ones_d[:].to_broadcast([128, 2]))
            nc.sync.dma_start(lna_t[:], lna_d[:].to_broadcast([128, D]))
            nc.sync.dma_start(lnb_t[:], lnb_d[:].to_broadcast([128, D]))

            for it in range(iters):
                P = f"it{it}_"

                evac_flip = [0]

                def evac(dst, src):
                    """PSUM->SBUF fp8 conversion, alternating DVE/ACT."""
                    evac_flip[0] ^= 1
                    if evac_flip[0]:
                        nc.vector.tensor_copy(dst, src)
                    else:
                        nc.scalar.activation(dst, src, AF.Copy)

                def qk_proj(hp, w8, ncols, dst2, name):
                    """Project K (ncols=L) or Q (ncols=HALF) for head-pair hp,
                    evacuate as fp8, then DMA re-layout into dst2's
                    [32, 2(dk-half), m] per-head layout."""
                    chunk = st.tile([128, ncols], F8, name=f"{P}{name}c_{hp}",
                                    tag=f"{name}stage")
                    for mc in range(ncols // 512):
                        ps = psQ.tile([128, 512], F32,
                                      name=f"{P}{name}ps_{hp}_{mc}", tag="ps")
                        for cc in range(2):
                            for j in range(4):
                                nc.tensor.matmul(
                                    ps[:, cc * 256:(cc + 1) * 256],
                                    w8[:, 2 * j:2 * j + 2,
                                       hp * 128:(hp + 1) * 128],
                                    qt8[:, 2 * j:2 * j + 2,
                                        mc * 512 + cc * 256:
                                        mc * 512 + (cc + 1) * 256],
                                    start=(j == 0), stop=(j == 3),
                                    perf_mode=PM.DoubleRow,
                                )
                        evac(chunk[:, mc * 512:(mc + 1) * 512], ps[:])
                    for h2 in range(2):
                        h = 2 * hp + h2
                        for t in range(2):
                            nc.sync.dma_start(
                                dst2[32 * (h % 3):32 * (h % 3) + 32,
                                     h // 3, t, :],
                                chunk[64 * h2 + 32 * t:64 * h2 + 32 * t + 32, :],
                            )

                def v_proj(mt, vhalf):
                    """V for m-tile mt, heads [8*vhalf, 8*vhalf+8)."""
                    ps = psQ.tile([128, 512], F32, name=f"{P}vps_{mt}_{vhalf}",
                                  tag="ps")
                    for g in range(4):
                        hp = vhalf * 4 + g
                        for j in range(4):
                            nc.tensor.matmul(
                                ps[:, g * 128:(g + 1) * 128],
                                qt8[:, 2 * j:2 * j + 2,
                                    mt * 128:(mt + 1) * 128],
                                wv8[:, 2 * j:2 * j + 2,
                                    hp * 128:(hp + 1) * 128],
                                start=(j == 0), stop=(j == 3),
                                perf_mode=PM.DoubleRow,
                            )
                    evac(vaug[:, mt, vhalf * 512:(vhalf + 1) * 512], ps[:])

                def attn(h):
                    a, b2 = h % 3, h // 3
                    eh = ehp.tile([128, 8, 2, HALF], F8, name=f"{P}eh_{h}",
                                  tag="eh")
                    for mt in range(16):
                        sp = psS.tile([128, HALF], F32,
                                      name=f"{P}sp_{h}_{mt}", tag="s")
                        for qc in range(4):
                            nc.tensor.matmul(
                                sp[:, qc * 256:(qc + 1) * 256],
                                kht2[32 * a:32 * a + 32, b2, :,
                                     mt * 128:(mt + 1) * 128],
                                qht2[32 * a:32 * a + 32, b2, :,
                                     qc * 256:(qc + 1) * 256],
                                start=True, stop=True,
                                perf_mode=PM.DoubleRow,
                            )
                        nc.scalar.activation(
                            eh[:, mt // 2, mt % 2, 0:EA], sp[:, 0:EA],
                            AF.Exp, scale=1.0 / TEMPER)
                        nc.vector._custom_dve(
                            EXP_OP, out=eh[:, mt // 2, mt % 2, EA:HALF],
                            in0=sp[:, EA:HALF],
                            s0=1.0 / (TEMPER * 8.0), s1=1.0, imm2=0.5)
                    den = psD.tile([128, 8], F32, name=f"{P}den_{h}", tag="den")
                    ot = psO.tile([128, 8, 64], F32, name=f"{P}ot_{h}", tag="ot")
                    for qt in range(8):
                        for p in range(8):
                            nc.tensor.matmul(
                                ot[:, qt, :],
                                eh[:, p, :, qt * 128:(qt + 1) * 128],
                                vaug[:, 2 * p:2 * p + 2, h * 64:(h + 1) * 64],
                                start=(p == 0), stop=(p == 7),
                                perf_mode=PM.DoubleRow,
                            )
                        for p in range(8):
                            nc.tensor.matmul(
                                den[:, qt:qt + 1],
                                eh[:, p, :, qt * 128:(qt + 1) * 128],
                                ones8[:, :, 0:1],
                                start=(p == 0), stop=(p == 7),
                                perf_mode=PM.DoubleRow,
                            )
                    recb = smp.tile([128, 8], F32, name=f"{P}rec_{h}", tag="recb")
                    nc.vector.reciprocal_approx_fast(recb[:], den[:])
                    for qt in range(8):
                        dst = ct[:, qt, h * 64:(h + 1) * 64]
                        if qt % 2 == 0:
                            nc.vector.tensor_scalar_mul(
                                dst, ot[:, qt, :], recb[:, qt:qt + 1])
                        else:
                            nc.scalar.activation(
                                dst, ot[:, qt, :], AF.Identity,
                                scale=recb[:, qt:qt + 1])

                # ---- emission schedule: overlap projections with attention --
                qk_proj(0, wq8, HALF, qht2, "q")
                qk_proj(0, wk8, L, kht2, "k")
                for mt in range(16):
                    v_proj(mt, 0)
                attn(0)
                attn(1)
                for i in range(1, 4):
                    qk_proj(i, wq8, HALF, qht2, "q")
                    qk_proj(i, wk8, L, kht2, "k")
                    attn(2 * i)
                    attn(2 * i + 1)
                for mt in range(16):
                    v_proj(mt, 1)
                for i in range(4, 8):
                    qk_proj(i, wq8, HALF, qht2, "q")
                    qk_proj(i, wk8, L, kht2, "k")
                    attn(2 * i)
                    attn(2 * i + 1)

                # ---- transpose CT, project, residual, layernorm -------------
                for qt in range(8):
                    ctb = st.tile([128, 8, 128], BF16, name=f"{P}ctb_{qt}",
                                  tag="ctb")
                    nc.sync.dma_start_transpose(ctb[:], ct[:, qt, :])
                    nc.gpsimd.tensor_copy(
                        ctT8[:, :, qt * 128:(qt + 1) * 128], ctb[:])
                for qt in range(8):
                    qr = st.tile([128, D], F32, name=f"{P}qr_{qt}", tag="qr")
                    nc.sync.dma_start(qr[:], qres_d[qt * 128:(qt + 1) * 128, :])
                    yt = st.tile([128, D], F32, name=f"{P}yt_{qt}", tag="yt")
                    for oc in range(2):
                        yp = psQ.tile([128, 512], F32,
                                      name=f"{P}yp_{qt}_{oc}", tag="ps")
                        for cc in range(2):
                            for j in range(4):
                                nc.tensor.matmul(
                                    yp[:, cc * 256:(cc + 1) * 256],
                                    ctT8[:, 2 * j:2 * j + 2,
                                         qt * 128:(qt + 1) * 128],
                                    pw8[:, 2 * j:2 * j + 2,
                                        oc * 512 + cc * 256:
                                        oc * 512 + (cc + 1) * 256],
                                    start=(j == 0), stop=(j == 3),
                                    perf_mode=PM.DoubleRow,
                                )
                        nc.vector.tensor_add(
                            yt[:, oc * 512:(oc + 1) * 512], yp[:],
                            qr[:, oc * 512:(oc + 1) * 512])
                    # layernorm: mu, sigma (ddof=1), (y-mu)/(sigma+eps)*a+b
                    o_t = st.tile([128, D], F32, name=f"{P}o_{qt}", tag="o")
                    s = smp.tile([128, 1], F32, name=f"{P}s_{qt}", tag="ln_s")
                    nc.vector.reduce_sum(s[:], yt[:], axis=AX.X)
                    negmean = smp.tile([128, 1], F32, name=f"{P}nm_{qt}",
                                       tag="ln_nm")
                    nc.vector.tensor_scalar_mul(negmean[:], s[:], -1.0 / D)
                    mean = smp.tile([128, 1], F32, name=f"{P}m_{qt}", tag="ln_m")
                    nc.vector.tensor_scalar_mul(mean[:], s[:], 1.0 / D)
                    ss = smp.tile([128, 1], F32, name=f"{P}ss_{qt}", tag="ln_ss")
                    ss2 = smp.tile([128, 1], F32, name=f"{P}s2_{qt}", tag="ln_s2")
                    for oc in range(2):
                        nc.scalar.activation(
                            o_t[:, oc * 512:(oc + 1) * 512],
                            yt[:, oc * 512:(oc + 1) * 512], AF.Square,
                            bias=negmean[:],
                            accum_out=(ss[:] if oc == 0 else ss2[:]),
                        )
                    nc.vector.tensor_add(ss[:], ss[:], ss2[:])
                    sigma = smp.tile([128, 1], F32, name=f"{P}sg_{qt}",
                                     tag="ln_sg")
                    nc.scalar.activation(sigma[:], ss[:], AF.Sqrt,
                                         scale=1.0 / (D - 1))
                    var = smp.tile([128, 1], F32, name=f"{P}var_{qt}",
                                   tag="ln_var")
                    nc.vector.tensor_scalar_mul(var[:], ss[:], 1.0 / (D - 1))
                    rs = smp.tile([128, 1], F32, name=f"{P}rs_{qt}", tag="ln_rs")
                    nc.vector.reciprocal(rs[:], sigma[:])
                    t1 = smp.tile([128, 1], F32, name=f"{P}t1_{qt}", tag="ln_t1")
                    nc.vector.tensor_mul(t1[:], var[:], rs[:])
                    nc.vector.tensor_add(t1[:], t1[:], sigma[:])
                    dd = smp.tile([128, 1], F32, name=f"{P}dd_{qt}", tag="ln_dd")
                    nc.vector.tensor_scalar(dd[:], t1[:], 0.5, LN_EPS,
                                            ALU.mult, ALU.add)
                    rec2 = smp.tile([128, 1], F32, name=f"{P}rc_{qt}",
                                    tag="ln_rc")
                    nc.vector.reciprocal(rec2[:], dd[:])
                    nc.gpsimd.tensor_scalar(o_t[:], yt[:], mean[:], rec2[:],
                                            ALU.subtract, ALU.mult)
                    nc.gpsimd.tensor_mul(o_t[:], o_t[:], lna_t[:])
                    nc.gpsimd.tensor_add(o_t[:], o_t[:], lnb_t[:])
                    nc.sync.dma_start(out_d[qt * 128:(qt + 1) * 128, :], o_t[:])

    nc.compile()
    return nc


def _get_nc():
    if "nc" not in _CACHE:
        _CACHE["nc"] = build()
    return _CACHE["nc"]


def _interleave8(a):
    """[D, N] f32 -> [128, 8, N] fp8 with [p, 2j+t, n] = a[j*256+t*128+p, n]."""
    n = a.shape[1]
    return np.ascontiguousarray(
        a.reshape(4, 2, 128, n).transpose(2, 0, 1, 3).reshape(128, 8, n)
    ).astype(NP8)


def _in_maps(q, w_qs, w_ks, w_vs, proj_w, proj_b, ln_a, ln_b):
    wq8 = _interleave8(np.ascontiguousarray(
        w_qs.transpose(1, 0, 2).reshape(D, H * DK)))
    wk8 = _interleave8(np.ascontiguousarray(
        w_ks.transpose(1, 0, 2).reshape(D, H * DK)))
    wv8 = _interleave8(np.ascontiguousarray(
        w_vs.transpose(1, 0, 2).reshape(D, H * DK)))
    pw8 = _interleave8(np.ascontiguousarray(proj_w.T))
    ones8 = np.ones((1, 2), NP8)
    lna = np.ascontiguousarray(ln_a[None, :]).astype(np.float32)
    lnb = np.ascontiguousarray(ln_b[None, :]).astype(np.float32)
    maps = []
    for c in range(8):
        b, half = c // 2, c % 2
        qb = q[b]
        perm = np.r_[half * HALF:(half + 1) * HALF,
                     (1 - half) * HALF:(2 - half) * HALF]
        qt8 = _interleave8(np.ascontiguousarray(qb.T[:, perm]))
        qres = np.ascontiguousarray(
            qb[half * HALF:(half + 1) * HALF, :] + proj_b[None, :]
        ).astype(np.float32)
        maps.append({
            "qt8": qt8, "qres": qres,
            "wq8": wq8, "wk8": wk8, "wv8": wv8, "pw8": pw8,
            "ones8": ones8, "lna": lna, "lnb": lnb,
        })
    return maps


def kernel(q, w_qs, w_ks, w_vs, proj_w, proj_b, ln_a, ln_b, **kw):
    q = np.asarray(q, dtype=np.float32)
    w_qs = np.asarray(w_qs, dtype=np.float32)
    w_ks = np.asarray(w_ks, dtype=np.float32)
    w_vs = np.asarray(w_vs, dtype=np.float32)
    proj_w = np.asarray(proj_w, dtype=np.float32)
    proj_b = np.asarray(proj_b, dtype=np.float32)
    ln_a = np.asarray(ln_a, dtype=np.float32)
    ln_b = np.asarray(ln_b, dtype=np.float32)

    in_maps = _in_maps(q, w_qs, w_ks, w_vs, proj_w, proj_b, ln_a, ln_b)
    nc = _get_nc()
    res = run_bass_kernel_spmd(nc, in_maps, core_ids=list(range(8))).results

    out = np.empty((B, L, D), dtype=np.float32)
    for c in range(8):
        b, half = c // 2, c % 2
        out[b, half * HALF:(half + 1) * HALF, :] = res[c]["out"]
    return out


# revision 13
# speedup vs baseline: 1.6809x; 1.0221x over previous
"""MultiHeadAttention TRN2 Bass kernel (8 NeuronCores), fp8 DoubleRow version.

Sharding: core c = (batch b = c//2, query-half = c%2). Each core computes
K/V for its full batch (2048 keys) and attention + output projection + LN
for its 1024 query rows. No collectives; host gathers per-core outputs.

Device math (fp8e4m3 matmuls in DoubleRow perf mode = 2 k-tiles of 128
contracted per instruction at 0.5 cycles per output column):
  Q/K proj: out [hd(2 heads x 64dk), m]  = w8[d,2,hd].T @ qt8[d,2,m]
  V proj:   out [m, 2 heads x 64]        = qt8[d,2,m].T @ wv8[d,2,hv]
  kht2/qht2: DMA re-layout to [32, 2(dk-half), m] per head so S can run
  DoubleRow with dk = 2x32.
  S[m, q]   = kht2[32,2,mtile].T @ qht2[32,2,q]      (per head, per m-tile)
  E = exp(S/32): split between ACT (Exp) and a custom DVE op
      (0.5*(1+x/8)^2+0.5)^8 ~ e^x with <1% typical error.
  O[q, 64]  = sum_p E[m,2,q].T @ V[m,2,64]           (E is the stationary)
  den[q]    = sum_p E[m,2,q].T @ ones[m,2,1]
  CT[q, (h,64)] = O * (1/den)    (per-partition scalar broadcast)
  ctT = DMA-transpose(CT bf16) -> fp8;  Y = ctT.T @ pw8 + residual; LayerNorm
"""
import numpy as np
import ml_dtypes

import concourse.bass as bass
import concourse.mybir as mybir
import concourse.tile as tile
from concourse import bacc
from concourse.bass_utils import run_bass_kernel_spmd

# ---------------- custom DVE exp op ----------------
import concourse.dve_ops as dve_ops
from concourse.dve_spec import Spec, Src0, C0, C1, C2, sq


def _exp_ref(in0, in1, c0, c1, c2):
    x = in0.astype(np.float32)
    t = ((x * c0 + c1) ** 2 * c2 + c2).astype(np.float32)
    t = (t * t).astype(np.float32)
    t = (t * t).astype(np.float32)
    return (t * t).astype(np.float32)


def _get_exp_op():
    name = "EXP_APPROX_ANT"
    for op in dve_ops.OPS:
        if op.name == name:
            return op
    body = sq(sq(sq(sq(Src0 * C0 + C1) * C2 + C2)))
    op = dve_ops.DveOp(
        name,
        Spec(body=body, reference=_exp_ref),
        subdim=False,
        uops_sha={"v3": "7cc04e385f99d2ac", "v4": "4c6dc6b0499997cd"},
    )
    row = max(dve_ops._SUB_OPCODE_FOR_NAME.values()) + 1
    assert row < 0x20
    dve_ops.OPS.append(op)
    dve_ops.CUSTOM_DVE_SPECS[name] = op.spec
    dve_ops._SUB_OPCODE_FOR_NAME[name] = row
    return op


EXP_OP = _get_exp_op()

F32 = mybir.dt.float32
BF16 = mybir.dt.bfloat16
F8 = mybir.dt.float8e4
AF = mybir.ActivationFunctionType
ALU = mybir.AluOpType
AX = mybir.AxisListType
PM = mybir.MatmulPerfMode

B, L, D = 4, 2048, 1024
H, DK = 16, 64
HALF = 1024
TEMPER = 32.0
LN_EPS = 1e-3
EA = 576  # columns of each 1024-wide exp tile handled by ACT (rest on DVE)
NP8 = ml_dtypes.float8_e4m3

_CACHE = {}


def build(iters=1):
    nc = bacc.Bacc(None, target_bir_lowering=False)
    qt8_d = nc.dram_tensor("qt8", [128, 8, L], F8, kind="ExternalInput")
    wq8_d = nc.dram_tensor("wq8", [128, 8, H * DK], F8, kind="ExternalInput")
    wk8_d = nc.dram_tensor("wk8", [128, 8, H * DK], F8, kind="ExternalInput")
    wv8_d = nc.dram_tensor("wv8", [128, 8, H * DK], F8, kind="ExternalInput")
    pw8_d = nc.dram_tensor("pw8", [128, 8, D], F8, kind="ExternalInput")
    ones_d = nc.dram_tensor("ones8", [1, 2], F8, kind="ExternalInput")
    qres_d = nc.dram_tensor("qres", [HALF, D], F32, kind="ExternalInput")
    lna_d = nc.dram_tensor("lna", [1, D], F32, kind="ExternalInput")
    lnb_d = nc.dram_tensor("lnb", [1, D], F32, kind="ExternalInput")
    out_d = nc.dram_tensor("out", [HALF, D], F32, kind="ExternalOutput")

    with tile.TileContext(nc) as tc:
        with (
            tc.tile_pool(name="c1", bufs=1) as c1,
            tc.tile_pool(name="st", bufs=2) as st,
            tc.tile_pool(name="eh", bufs=2) as ehp,
            tc.tile_pool(name="sm", bufs=2) as smp,
            tc.tile_pool(name="psQ", bufs=2, space="PSUM") as psQ,
            tc.tile_pool(name="psD", bufs=1, space="PSUM") as psD,
            tc.tile_pool(name="psS", bufs=2, space="PSUM") as psS,
            tc.tile_pool(name="psO", bufs=1, space="PSUM") as psO,
        ):
            # persistent SBUF
            qt8 = c1.tile([128, 8, L], F8, name="qt8_t")
            wq8 = c1.tile([128, 8, H * DK], F8, name="wq8_t")
            wk8 = c1.tile([128, 8, H * DK], F8, name="wk8_t")
            wv8 = c1.tile([128, 8, H * DK], F8, name="wv8_t")
            pw8 = c1.tile([128, 8, D], F8, name="pw8_t")
            ones8 = c1.tile([128, 2, 1], F8, name="ones8_t")
            kht2 = c1.tile([128, 6, 2, L], F8, name="kht2_t")
            qht2 = c1.tile([128, 6, 2, HALF], F8, name="qht2_t")
            vaug = c1.tile([128, 16, H * DK], F8, name="vaug_t")
            ctTb = c1.tile([128, 8, D], BF16, name="ctTb_t")
            ctT8 = c1.tile([128, 8, D], F8, name="ctT8_t")
            lna_t = c1.tile([128, D], F32, name="lna_t")
            lnb_t = c1.tile([128, D], F32, name="lnb_t")

            nc.sync.dma_start(wk8[:], wk8_d[:])
            nc.sync.dma_start(wq8[:], wq8_d[:])
            for mc4 in range(4):
                nc.sync.dma_start(qt8[:, :, mc4 * 512:(mc4 + 1) * 512],
                                  qt8_d[:, :, mc4 * 512:(mc4 + 1) * 512])
            nc.sync.dma_start(wv8[:], wv8_d[:])
            nc.sync.dma_start(ones8[:, :, 0], ones_d[:].to_broadcast([128, 2]))

            for it in range(iters):
                P = f"it{it}_"

                evac_flip = [0]

                def evac(dst, src):
                    """PSUM->SBUF fp8 conversion, alternating DVE/ACT."""
                    evac_flip[0] ^= 1
                    if evac_flip[0]:
                        nc.vector.tensor_copy(dst, src)
                    else:
                        nc.scalar.activation(dst, src, AF.Copy)

                def qk_proj(hp, w8, ncols, dst2, name):
                    """Project K (ncols=L) or Q (ncols=HALF) for head-pair hp,
                    evacuate as fp8, then DMA re-layout into dst2's
                    [32, 2(dk-half), m] per-head layout."""
                    chunk = st.tile([128, ncols], F8, name=f"{P}{name}c_{hp}",
                                    tag=f"{name}stage")
                    for mc in range(ncols // 512):
                        ps = psQ.tile([128, 512], F32,
                                      name=f"{P}{name}ps_{hp}_{mc}", tag="ps")
                        for cc in range(2):
                            for j in range(4):
                                nc.tensor.matmul(
                                    ps[:, cc * 256:(cc + 1) * 256],
                                    w8[:, 2 * j:2 * j + 2,
                                       hp * 128:(hp + 1) * 128],
                                    qt8[:, 2 * j:2 * j + 2,
                                        mc * 512 + cc * 256:
                                        mc * 512 + (cc + 1) * 256],
                                    start=(j == 0), stop=(j == 3),
                                    perf_mode=PM.DoubleRow,
                                )
                        evac(chunk[:, mc * 512:(mc + 1) * 512], ps[:])
                    for h2 in range(2):
                        h = 2 * hp + h2
                        for t in range(2):
                            nc.sync.dma_start(
                                dst2[32 * (h % 3):32 * (h % 3) + 32,
                                     h // 3, t, :],
                                chunk[64 * h2 + 32 * t:64 * h2 + 32 * t + 32, :],
                            )

                def v_proj(mt, vhalf):
                    """V for m-tile mt, heads [8*vhalf, 8*vhalf+8)."""
                    ps = psQ.tile([128, 512], F32, name=f"{P}vps_{mt}_{vhalf}",
                                  tag="ps")
                    for g in range(4):
                        hp = vhalf * 4 + g
                        for j in range(4):
                            nc.tensor.matmul(
                                ps[:, g * 128:(g + 1) * 128],
                                qt8[:, 2 * j:2 * j + 2,
                                    mt * 128:(mt + 1) * 128],
                                wv8[:, 2 * j:2 * j + 2,
                                    hp * 128:(hp + 1) * 128],
                                start=(j == 0), stop=(j == 3),
                                perf_mode=PM.DoubleRow,
                            )
                    evac(vaug[:, mt, vhalf * 512:(vhalf + 1) * 512], ps[:])

                ehs = {}
                ots = {}
                dens = {}
                recbs = {}

                def attn_ov(h):
                    """O + den matmuls for head h (PE), emitted one head late
                    so the division pipeline never blocks the engines."""
                    eh, ot, den = ehs[h], ots[h], dens[h]
                    for qt in range(8):
                        for p in range(8):
                            nc.tensor.matmul(
                                ot[:, qt, :],
                                eh[:, p, :, qt * 128:(qt + 1) * 128],
                                vaug[:, 2 * p:2 * p + 2, h * 64:(h + 1) * 64],
                                start=(p == 0), stop=(p == 7),
                                perf_mode=PM.DoubleRow,
                            )
                        for p in range(8):
                            nc.tensor.matmul(
                                den[:, qt:qt + 1],
                                eh[:, p, :, qt * 128:(qt + 1) * 128],
                                ones8[:, :, 0:1],
                                start=(p == 0), stop=(p == 7),
                                perf_mode=PM.DoubleRow,
                            )

                def attn_recb(h):
                    recb = smp.tile([128, 8], F32, name=f"{P}rec_{h}",
                                    tag="recb")
                    recbs[h] = recb
                    nc.vector.reciprocal_approx_fast(recb[:], dens[h][:])

                cbs = {}

                def attn_div(h):
                    """CT block = O/den (bf16) into a 4-head staging block;
                    transpose into ctTb once per quad."""
                    ot, recb = ots[h], recbs[h]
                    for qt in range(8):
                        if h % 4 == 0:
                            cbs[qt] = st.tile([128, 4, 64], BF16,
                                              name=f"{P}cb_{h // 4}_{qt}",
                                              tag="ctblk", bufs=9)
                        cb = cbs[qt]
                        if qt % 2 == 0:
                            nc.vector.tensor_scalar_mul(
                                cb[:, h % 4, :], ot[:, qt, :],
                                recb[:, qt:qt + 1])
                        else:
                            nc.scalar.activation(
                                cb[:, h % 4, :], ot[:, qt, :], AF.Identity,
                                scale=recb[:, qt:qt + 1])
                        if h % 4 == 3:
                            nc.sync.dma_start_transpose(
                                ctTb[:, 2 * (h // 4):2 * (h // 4) + 2,
                                     qt * 128:(qt + 1) * 128],
                                cb[:, :, :])

                def attn_sx(h):
                    """S matmuls + exp for head h; interleaves the previous
                    head's O/den/recb/div at stall-free points."""
                    a, b2 = h % 3, h // 3
                    eh = ehp.tile([128, 8, 2, HALF], F8, name=f"{P}eh_{h}",
                                  tag="eh")
                    ehs[h] = eh
                    ots[h] = psO.tile([128, 8, 64], F32, name=f"{P}ot_{h}",
                                      tag="ot")
                    dens[h] = psD.tile([128, 8], F32, name=f"{P}den_{h}",
                                       tag="den")
                    if h > 0:
                        attn_ov(h - 1)
                    for mt in range(16):
                        sp = psS.tile([128, HALF], F32,
                                      name=f"{P}sp_{h}_{mt}", tag="s")
                        for qc in range(4):
                            nc.tensor.matmul(
                                sp[:, qc * 256:(qc + 1) * 256],
                                kht2[32 * a:32 * a + 32, b2, :,
                                     mt * 128:(mt + 1) * 128],
                                qht2[32 * a:32 * a + 32, b2, :,
                                     qc * 256:(qc + 1) * 256],
                                start=True, stop=True,
                                perf_mode=PM.DoubleRow,
                            )
                        nc.scalar.activation(
                            eh[:, mt // 2, mt % 2, 0:EA], sp[:, 0:EA],
                            AF.Exp, scale=1.0 / TEMPER)
                        nc.vector._custom_dve(
                            EXP_OP, out=eh[:, mt // 2, mt % 2, EA:HALF],
                            in0=sp[:, EA:HALF],
                            s0=1.0 / (TEMPER * 8.0), s1=1.0, imm2=0.5)
                        if mt == 1 and h > 0:
                            attn_recb(h - 1)
                    if h > 0:
                        attn_div(h - 1)

                def attn_last(h):
                    attn_ov(h)
                    attn_recb(h)
                    attn_div(h)

                # ---- emission schedule: overlap projections with attention --
                qk_proj(0, wq8, HALF, qht2, "q")
                qk_proj(0, wk8, L, kht2, "k")
                for mt in range(16):
                    v_proj(mt, 0)
                attn_sx(0)
                attn_sx(1)
                for i in range(1, 4):
                    qk_proj(i, wq8, HALF, qht2, "q")
                    qk_proj(i, wk8, L, kht2, "k")
                    attn_sx(2 * i)
                    attn_sx(2 * i + 1)
                for mt in range(16):
                    v_proj(mt, 1)
                nc.sync.dma_start(pw8[:], pw8_d[:])
                nc.sync.dma_start(lna_t[:], lna_d[:].to_broadcast([128, D]))
                nc.sync.dma_start(lnb_t[:], lnb_d[:].to_broadcast([128, D]))
                for i in range(4, 8):
                    qk_proj(i, wq8, HALF, qht2, "q")
                    qk_proj(i, wk8, L, kht2, "k")
                    attn_sx(2 * i)
                    attn_sx(2 * i + 1)
                attn_last(15)

                # ---- convert ctTb -> fp8, project, residual, layernorm ------
                for qt in range(8):
                    nc.gpsimd.tensor_copy(
                        ctT8[:, :, qt * 128:(qt + 1) * 128],
                        ctTb[:, :, qt * 128:(qt + 1) * 128])
                for qt in range(8):
                    qr = st.tile([128, D], F32, name=f"{P}qr_{qt}", tag="qr")
                    nc.sync.dma_start(qr[:], qres_d[qt * 128:(qt + 1) * 128, :])
                    yt = st.tile([128, D], F32, name=f"{P}yt_{qt}", tag="yt")
                    for oc in range(2):
                        yp = psQ.tile([128, 512], F32,
                                      name=f"{P}yp_{qt}_{oc}", tag="ps")
                        for cc in range(2):
                            for j in range(4):
                                nc.tensor.matmul(
                                    yp[:, cc * 256:(cc + 1) * 256],
                                    ctT8[:, 2 * j:2 * j + 2,
                                         qt * 128:(qt + 1) * 128],
                                    pw8[:, 2 * j:2 * j + 2,
                                        oc * 512 + cc * 256:
                                        oc * 512 + (cc + 1) * 256],
                                    start=(j == 0), stop=(j == 3),
                                    perf_mode=PM.DoubleRow,
                                )
                        nc.vector.tensor_add(
                            yt[:, oc * 512:(oc + 1) * 512], yp[:],
                            qr[:, oc * 512:(oc + 1) * 512])
                    # layernorm: mu, sigma (ddof=1), (y-mu)/(sigma+eps)*a+b
                    o_t = st.tile([128, D], F32, name=f"{P}o_{qt}", tag="o")
                    s = smp.tile([128, 1], F32, name=f"{P}s_{qt}", tag="ln_s")
                    nc.vector.reduce_sum(s[:], yt[:], axis=AX.X)
                    negmean = smp.tile([128, 1], F32, name=f"{P}nm_{qt}",
                                       tag="ln_nm")
                    nc.vector.tensor_scalar_mul(negmean[:], s[:], -1.0 / D)
                    mean = smp.tile([128, 1], F32, name=f"{P}m_{qt}", tag="ln_m")
                    nc.vector.tensor_scalar_mul(mean[:], s[:], 1.0 / D)
                    ss = smp.tile([128, 1], F32, name=f"{P}ss_{qt}", tag="ln_ss")
                    ss2 = smp.tile([128, 1], F32, name=f"{P}s2_{qt}", tag="ln_s2")
                    for oc in range(2):
                        nc.scalar.activation(
                            o_t[:, oc * 512:(oc + 1) * 512],
                            yt[:, oc * 512:(oc + 1) * 512], AF.Square,
                            bias=negmean[:],
                            accum_out=(ss[:] if oc == 0 else ss2[:]),
                        )
                    nc.vector.tensor_add(ss[:], ss[:], ss2[:])
                    sigma = smp.tile([128, 1], F32, name=f"{P}sg_{qt}",
                                     tag="ln_sg")
                    nc.scalar.activation(sigma[:], ss[:], AF.Sqrt,
                                         scale=1.0 / (D - 1))
                    var = smp.tile([128, 1], F32, name=f"{P}var_{qt}",
                                   tag="ln_var")
                    nc.vector.tensor_scalar_mul(var[:], ss[:], 1.0 / (D - 1))
                    rs = smp.tile([128, 1], F32, name=f"{P}rs_{qt}", tag="ln_rs")
                    nc.vector.reciprocal(rs[:], sigma[:])
                    t1 = smp.tile([128, 1], F32, name=f"{P}t1_{qt}", tag="ln_t1")
                    nc.vector.tensor_mul(t1[:], var[:], rs[:])
                    nc.vector.tensor_add(t1[:], t1[:], sigma[:])
                    dd = smp.tile([128, 1], F32, name=f"{P}dd_{qt}", tag="ln_dd")
                    nc.vector.tensor_scalar(dd[:], t1[:], 0.5, LN_EPS,
                                            ALU.mult, ALU.add)
                    rec2 = smp.tile([128, 1], F32, name=f"{P}rc_{qt}",
                                    tag="ln_rc")
                    nc.vector.reciprocal(rec2[:], dd[:])
                    nc.gpsimd.tensor_scalar(o_t[:], yt[:], mean[:], rec2[:],
                                            ALU.subtract, ALU.mult)
                    nc.gpsimd.tensor_mul(o_t[:], o_t[:], lna_t[:])
                    nc.gpsimd.tensor_add(o_t[:], o_t[:], lnb_t[:])
                    nc.sync.dma_start(out_d[qt * 128:(qt + 1) * 128, :], o_t[:])

    nc.compile()
    return nc


def _get_nc():
    if "nc" not in _CACHE:
        _CACHE["nc"] = build()
    return _CACHE["nc"]


def _interleave8(a):
    """[D, N] f32 -> [128, 8, N] fp8 with [p, 2j+t, n] = a[j*256+t*128+p, n]."""
    n = a.shape[1]
    return np.ascontiguousarray(
        a.reshape(4, 2, 128, n).transpose(2, 0, 1, 3).reshape(128, 8, n)
    ).astype(NP8)


def _in_maps(q, w_qs, w_ks, w_vs, proj_w, proj_b, ln_a, ln_b):
    wq8 = _interleave8(np.ascontiguousarray(
        w_qs.transpose(1, 0, 2).reshape(D, H * DK)))
    wk8 = _interleave8(np.ascontiguousarray(
        w_ks.transpose(1, 0, 2).reshape(D, H * DK)))
    wv8 = _interleave8(np.ascontiguousarray(
        w_vs.transpose(1, 0, 2).reshape(D, H * DK)))
    pw8 = _interleave8(np.ascontiguousarray(proj_w.T))
    ones8 = np.ones((1, 2), NP8)
    lna = np.ascontiguousarray(ln_a[None, :]).astype(np.float32)
    lnb = np.ascontiguousarray(ln_b[None, :]).astype(np.float32)
    maps = []
    for c in range(8):
        b, half = c // 2, c % 2
        qb = q[b]
        perm = np.r_[half * HALF:(half + 1) * HALF,
                     (1 - half) * HALF:(2 - half) * HALF]
        qt8 = _interleave8(np.ascontiguousarray(qb.T[:, perm]))
        qres = np.ascontiguousarray(
            qb[half * HALF:(half + 1) * HALF, :] + proj_b[None, :]
        ).astype(np.float32)
        maps.append({
            "qt8": qt8, "qres": qres,
            "wq8": wq8, "wk8": wk8, "wv8": wv8, "pw8": pw8,
            "ones8": ones8, "lna": lna, "lnb": lnb,
        })
    return maps


def kernel(q, w_qs, w_ks, w_vs, proj_w, proj_b, ln_a, ln_b, **kw):
    q = np.asarray(q, dtype=np.float32)
    w_qs = np.asarray(w_qs, dtype=np.float32)
    w_ks = np.asarray(w_ks, dtype=np.float32)
    w_vs = np.asarray(w_vs, dtype=np.float32)
    proj_w = np.asarray(proj_w, dtype=np.float32)
    proj_b = np.asarray(proj_b, dtype=np.float32)
    ln_a = np.asarray(ln_a, dtype=np.float32)
    ln_b = np.asarray(ln_b, dtype=np.float32)

    in_maps = _in_maps(q, w_qs, w_ks, w_vs, proj_w, proj_b, ln_a, ln_b)
    nc = _get_nc()
    res = run_bass_kernel_spmd(nc, in_maps, core_ids=list(range(8))).results

    out = np.empty((B, L, D), dtype=np.float32)
    for c in range(8):
        b, half = c // 2, c % 2
        out[b, half * HALF:(half + 1) * HALF, :] = res[c]["out"]
    return out


# revision 15
# speedup vs baseline: 1.8909x; 1.1249x over previous
"""MultiHeadAttention TRN2 Bass kernel (8 NeuronCores), fp8 DoubleRow version.

Sharding: core c = (batch b = c//2, query-half = c%2). Each core computes
K/V for its full batch (2048 keys) and attention + output projection + LN
for its 1024 query rows. No collectives; host gathers per-core outputs.

Device math (fp8e4m3 matmuls in DoubleRow perf mode = 2 k-tiles of 128
contracted per instruction at 0.5 cycles per output column):
  Q/K proj: out [hd(2 heads x 64dk), m]  = w8[d,2,hd].T @ qt8[d,2,m]
  V proj:   out [m, 2 heads x 64]        = qt8[d,2,m].T @ wv8[d,2,hv]
  kht2/qht2: DMA re-layout to [32, 2(dk-half), m] per head so S can run
  DoubleRow with dk = 2x32.
  S[m, q]   = kht2[32,2,mtile].T @ qht2[32,2,q]      (per head, per m-tile)
  E = exp(S/32): split between ACT (Exp) and a custom DVE op
      (0.5*(1+x/8)^2+0.5)^8 ~ e^x with <1% typical error.
  O[q, 64]  = sum_p E[m,2,q].T @ V[m,2,64]           (E is the stationary)
  den[q]    = sum_p E[m,2,q].T @ ones[m,2,1]
  CT[q, (h,64)] = O * (1/den)    (per-partition scalar broadcast)
  ctT = DMA-transpose(CT bf16) -> fp8;  Y = ctT.T @ pw8 + residual; LayerNorm
"""
import numpy as np
import ml_dtypes

import concourse.bass as bass
import concourse.mybir as mybir
import concourse.tile as tile
from concourse import bacc
from concourse.bass_utils import run_bass_kernel_spmd

# ---------------- custom DVE exp op ----------------
import concourse.dve_ops as dve_ops
from concourse.dve_spec import Spec, Src0, C0, C1, C2, sq


def _exp_ref(in0, in1, c0, c1, c2):
    x = in0.astype(np.float32)
    t = ((x * c0 + c1) ** 2 * c2 + c2).astype(np.float32)
    t = (t * t).astype(np.float32)
    t = (t * t).astype(np.float32)
    return (t * t).astype(np.float32)


def _get_exp_op():
    name = "EXP_APPROX_ANT"
    for op in dve_ops.OPS:
        if op.name == name:
            return op
    body = sq(sq(sq(sq(Src0 * C0 + C1) * C2 + C2)))
    op = dve_ops.DveOp(
        name,
        Spec(body=body, reference=_exp_ref),
        subdim=False,
        uops_sha={"v3": "7cc04e385f99d2ac", "v4": "4c6dc6b0499997cd"},
    )
    row = max(dve_ops._SUB_OPCODE_FOR_NAME.values()) + 1
    assert row < 0x20
    dve_ops.OPS.append(op)
    dve_ops.CUSTOM_DVE_SPECS[name] = op.spec
    dve_ops._SUB_OPCODE_FOR_NAME[name] = row
    return op


EXP_OP = _get_exp_op()

F32 = mybir.dt.float32
BF16 = mybir.dt.bfloat16
F8 = mybir.dt.float8e4
AF = mybir.ActivationFunctionType
ALU = mybir.AluOpType
AX = mybir.AxisListType
PM = mybir.MatmulPerfMode

B, L, D = 4, 2048, 1024
H, DK = 16, 64
HALF = 1024
TEMPER = 32.0
LN_EPS = 1e-3
EA = 576  # columns of each 1024-wide exp tile handled by ACT (rest on DVE)
NP8 = ml_dtypes.float8_e4m3

_CACHE = {}


def build(iters=1):
    nc = bacc.Bacc(None, target_bir_lowering=False)
    qt8_d = nc.dram_tensor("qt8", [128, 8, L], F8, kind="ExternalInput")
    wq8_d = nc.dram_tensor("wq8", [128, 8, H * DK], F8, kind="ExternalInput")
    wk8_d = nc.dram_tensor("wk8", [128, 8, H * DK], F8, kind="ExternalInput")
    wv8_d = nc.dram_tensor("wv8", [128, 8, H * DK], F8, kind="ExternalInput")
    pw8_d = nc.dram_tensor("pw8", [128, 8, D], F8, kind="ExternalInput")
    ones_d = nc.dram_tensor("ones8", [1, 2], F8, kind="ExternalInput")
    qres_d = nc.dram_tensor("qres", [HALF, D], F32, kind="ExternalInput")
    lna_d = nc.dram_tensor("lna", [1, D], F32, kind="ExternalInput")
    lnb_d = nc.dram_tensor("lnb", [1, D], F32, kind="ExternalInput")
    out_d = nc.dram_tensor("out", [HALF, D], F32, kind="ExternalOutput")

    with tile.TileContext(nc) as tc:
        with (
            tc.tile_pool(name="c1", bufs=1) as c1,
            tc.tile_pool(name="st", bufs=2) as st,
            tc.tile_pool(name="eh", bufs=2) as ehp,
            tc.tile_pool(name="sm", bufs=2) as smp,
            tc.tile_pool(name="psQ", bufs=2, space="PSUM") as psQ,
            tc.tile_pool(name="psD", bufs=1, space="PSUM") as psD,
            tc.tile_pool(name="psSA", bufs=2, space="PSUM") as psSA,
            tc.tile_pool(name="psSB", bufs=2, space="PSUM") as psSB,
            tc.tile_pool(name="psO", bufs=1, space="PSUM") as psO,
        ):
            # persistent SBUF
            qt8 = c1.tile([128, 8, L], F8, name="qt8_t")
            wq8 = c1.tile([128, 8, H * DK], F8, name="wq8_t")
            wk8 = c1.tile([128, 8, H * DK], F8, name="wk8_t")
            wv8 = c1.tile([128, 8, H * DK], F8, name="wv8_t")
            pw8 = c1.tile([128, 8, D], F8, name="pw8_t")
            ones8 = c1.tile([128, 2, 1], F8, name="ones8_t")
            kht2 = c1.tile([128, 6, 2, L], F8, name="kht2_t")
            qht2 = c1.tile([128, 6, 2, HALF], F8, name="qht2_t")
            vaug = c1.tile([128, 16, H * DK], F8, name="vaug_t")
            ctTb = c1.tile([128, 8, D], BF16, name="ctTb_t")
            ctT8 = c1.tile([128, 8, D], F8, name="ctT8_t")
            lna_t = c1.tile([128, D], F32, name="lna_t")
            lnb_t = c1.tile([128, D], F32, name="lnb_t")

            nc.sync.dma_start(wk8[:], wk8_d[:])
            nc.sync.dma_start(wq8[:], wq8_d[:])
            for mc4 in range(4):
                nc.sync.dma_start(qt8[:, :, mc4 * 512:(mc4 + 1) * 512],
                                  qt8_d[:, :, mc4 * 512:(mc4 + 1) * 512])
            nc.sync.dma_start(wv8[:], wv8_d[:])
            nc.sync.dma_start(ones8[:, :, 0], ones_d[:].to_broadcast([128, 2]))

            for it in range(iters):
                P = f"it{it}_"

                evac_flip = [0]

                def evac(dst, src):
                    """PSUM->SBUF fp8 conversion, alternating DVE/ACT."""
                    evac_flip[0] ^= 1
                    if evac_flip[0]:
                        nc.vector.tensor_copy(dst, src)
                    else:
                        nc.scalar.activation(dst, src, AF.Copy)

                def qk_proj(hp, w8, ncols, dst2, name):
                    """Project K (ncols=L) or Q (ncols=HALF) for head-pair hp,
                    evacuate as fp8, then DMA re-layout into dst2's
                    [32, 2(dk-half), m] per-head layout."""
                    chunk = st.tile([128, ncols], F8, name=f"{P}{name}c_{hp}",
                                    tag=f"{name}stage")
                    for mc in range(ncols // 512):
                        ps = psQ.tile([128, 512], F32,
                                      name=f"{P}{name}ps_{hp}_{mc}", tag="ps")
                        for cc in range(2):
                            for j in range(4):
                                nc.tensor.matmul(
                                    ps[:, cc * 256:(cc + 1) * 256],
                                    w8[:, 2 * j:2 * j + 2,
                                       hp * 128:(hp + 1) * 128],
                                    qt8[:, 2 * j:2 * j + 2,
                                        mc * 512 + cc * 256:
                                        mc * 512 + (cc + 1) * 256],
                                    start=(j == 0), stop=(j == 3),
                                    perf_mode=PM.DoubleRow,
                                )
                        evac(chunk[:, mc * 512:(mc + 1) * 512], ps[:])
                    for h2 in range(2):
                        h = 2 * hp + h2
                        for t in range(2):
                            nc.sync.dma_start(
                                dst2[32 * (h % 3):32 * (h % 3) + 32,
                                     h // 3, t, :],
                                chunk[64 * h2 + 32 * t:64 * h2 + 32 * t + 32, :],
                            )

                def v_proj(mt, vhalf):
                    """V for m-tile mt, heads [8*vhalf, 8*vhalf+8)."""
                    ps = psQ.tile([128, 512], F32, name=f"{P}vps_{mt}_{vhalf}",
                                  tag="ps")
                    for g in range(4):
                        hp = vhalf * 4 + g
                        for j in range(4):
                            nc.tensor.matmul(
                                ps[:, g * 128:(g + 1) * 128],
                                qt8[:, 2 * j:2 * j + 2,
                                    mt * 128:(mt + 1) * 128],
                                wv8[:, 2 * j:2 * j + 2,
                                    hp * 128:(hp + 1) * 128],
                                start=(j == 0), stop=(j == 3),
                                perf_mode=PM.DoubleRow,
                            )
                    evac(vaug[:, mt, vhalf * 512:(vhalf + 1) * 512], ps[:])

                ehs = {}
                ots = {}
                dens = {}
                recbs = {}

                def attn_ov(h):
                    """O + den matmuls for head h (PE), emitted one head late
                    so the division pipeline never blocks the engines."""
                    eh, ot, den = ehs[h], ots[h], dens[h]
                    for qt in range(8):
                        for p in range(8):
                            nc.tensor.matmul(
                                ot[:, qt, :],
                                eh[:, p, :, qt * 128:(qt + 1) * 128],
                                vaug[:, 2 * p:2 * p + 2, h * 64:(h + 1) * 64],
                                start=(p == 0), stop=(p == 7),
                                perf_mode=PM.DoubleRow,
                            )
                        for p in range(8):
                            nc.tensor.matmul(
                                den[:, qt:qt + 1],
                                eh[:, p, :, qt * 128:(qt + 1) * 128],
                                ones8[:, :, 0:1],
                                start=(p == 0), stop=(p == 7),
                                perf_mode=PM.DoubleRow,
                            )

                def attn_recb(h):
                    recb = smp.tile([128, 8], F32, name=f"{P}rec_{h}",
                                    tag="recb")
                    recbs[h] = recb
                    nc.vector.reciprocal_approx_fast(recb[:], dens[h][:])

                cbs = {}

                def attn_div(h):
                    """CT block = O/den (bf16) into a 4-head staging block;
                    transpose into ctTb once per quad."""
                    ot, recb = ots[h], recbs[h]
                    for qt in range(8):
                        if h % 4 == 0:
                            cbs[qt] = st.tile([128, 4, 64], BF16,
                                              name=f"{P}cb_{h // 4}_{qt}",
                                              tag="ctblk", bufs=9)
                        cb = cbs[qt]
                        if qt % 2 == 0:
                            nc.vector.tensor_scalar_mul(
                                cb[:, h % 4, :], ot[:, qt, :],
                                recb[:, qt:qt + 1])
                        else:
                            nc.scalar.activation(
                                cb[:, h % 4, :], ot[:, qt, :], AF.Identity,
                                scale=recb[:, qt:qt + 1])
                        if h % 4 == 3:
                            nc.sync.dma_start_transpose(
                                ctTb[:, 2 * (h // 4):2 * (h // 4) + 2,
                                     qt * 128:(qt + 1) * 128],
                                cb[:, :, :])

                def attn_sx(h):
                    """S matmuls + exp for head h; interleaves the previous
                    head's O/den/recb/div at stall-free points."""
                    a, b2 = h % 3, h // 3
                    eh = ehp.tile([128, 8, 2, HALF], F8, name=f"{P}eh_{h}",
                                  tag="eh")
                    ehs[h] = eh
                    ots[h] = psO.tile([128, 8, 64], F32, name=f"{P}ot_{h}",
                                      tag="ot")
                    dens[h] = psD.tile([128, 8], F32, name=f"{P}den_{h}",
                                       tag="den")
                    if h > 0:
                        attn_ov(h - 1)
                    for mt in range(16):
                        spa = psSA.tile([128, 512], F32,
                                        name=f"{P}spa_{h}_{mt}", tag="sa")
                        spb = psSB.tile([128, 512], F32,
                                        name=f"{P}spb_{h}_{mt}", tag="sb")
                        for qc in range(4):
                            sp = spa if qc < 2 else spb
                            nc.tensor.matmul(
                                sp[:, (qc % 2) * 256:(qc % 2 + 1) * 256],
                                kht2[32 * a:32 * a + 32, b2, :,
                                     mt * 128:(mt + 1) * 128],
                                qht2[32 * a:32 * a + 32, b2, :,
                                     qc * 256:(qc + 1) * 256],
                                start=True, stop=True,
                                perf_mode=PM.DoubleRow,
                            )
                        nc.scalar.activation(
                            eh[:, mt // 2, mt % 2, 0:512], spa[:],
                            AF.Exp, scale=1.0 / TEMPER)
                        nc.vector._custom_dve(
                            EXP_OP, out=eh[:, mt // 2, mt % 2, 512:HALF],
                            in0=spb[:],
                            s0=1.0 / (TEMPER * 8.0), s1=1.0, imm2=0.5)
                        if mt == 1 and h > 0:
                            attn_recb(h - 1)
                    if h > 0:
                        attn_div(h - 1)

                def attn_last(h):
                    attn_ov(h)
                    attn_recb(h)
                    attn_div(h)

                # ---- emission schedule: overlap projections with attention --
                qk_proj(0, wq8, HALF, qht2, "q")
                qk_proj(0, wk8, L, kht2, "k")
                attn_sx(0)
                for mt in range(16):
                    v_proj(mt, 0)
                attn_sx(1)
                for i in range(1, 4):
                    qk_proj(i, wq8, HALF, qht2, "q")
                    qk_proj(i, wk8, L, kht2, "k")
                    attn_sx(2 * i)
                    attn_sx(2 * i + 1)
                for mt in range(16):
                    v_proj(mt, 1)
                nc.sync.dma_start(pw8[:], pw8_d[:])
                nc.sync.dma_start(lna_t[:], lna_d[:].to_broadcast([128, D]))
                nc.sync.dma_start(lnb_t[:], lnb_d[:].to_broadcast([128, D]))
                for i in range(4, 8):
                    qk_proj(i, wq8, HALF, qht2, "q")
                    qk_proj(i, wk8, L, kht2, "k")
                    attn_sx(2 * i)
                    attn_sx(2 * i + 1)
                attn_last(15)

                # ---- convert ctTb -> fp8, project, residual, layernorm ------
                for qt in range(8):
                    nc.gpsimd.tensor_copy(
                        ctT8[:, :, qt * 128:(qt + 1) * 128],
                        ctTb[:, :, qt * 128:(qt + 1) * 128])
                for qt in range(8):
                    qr = st.tile([128, D], F32, name=f"{P}qr_{qt}", tag="qr")
                    nc.sync.dma_start(qr[:], qres_d[qt * 128:(qt + 1) * 128, :])
                    yt = st.tile([128, D], F32, name=f"{P}yt_{qt}", tag="yt")
                    for oc in range(2):
                        yp = psQ.tile([128, 512], F32,
                                      name=f"{P}yp_{qt}_{oc}", tag="ps")
                        for cc in range(2):
                            for j in range(4):
                                nc.tensor.matmul(
                                    yp[:, cc * 256:(cc + 1) * 256],
                                    ctT8[:, 2 * j:2 * j + 2,
                                         qt * 128:(qt + 1) * 128],
                                    pw8[:, 2 * j:2 * j + 2,
                                        oc * 512 + cc * 256:
                                        oc * 512 + (cc + 1) * 256],
                                    start=(j == 0), stop=(j == 3),
                                    perf_mode=PM.DoubleRow,
                                )
                        nc.vector.tensor_add(
                            yt[:, oc * 512:(oc + 1) * 512], yp[:],
                            qr[:, oc * 512:(oc + 1) * 512])
                    # layernorm: mu, sigma (ddof=1), (y-mu)/(sigma+eps)*a+b
                    # stats via bn_stats/bn_aggr (one DVE pass for mean+var)
                    o_t = st.tile([128, D], F32, name=f"{P}o_{qt}", tag="o")
                    bst = smp.tile([128, 2, 6], F32, name=f"{P}bst_{qt}",
                                   tag="ln_bst")
                    for oc in range(2):
                        nc.vector.bn_stats(bst[:, oc, :],
                                           yt[:, oc * 512:(oc + 1) * 512])
                    mv = smp.tile([128, 2], F32, name=f"{P}mv_{qt}", tag="ln_mv")
                    nc.vector.bn_aggr(mv[:], bst[:])
                    sigma = smp.tile([128, 1], F32, name=f"{P}sg_{qt}",
                                     tag="ln_sg")
                    nc.scalar.activation(sigma[:], mv[:, 1:2], AF.Sqrt,
                                         scale=float(D) / (D - 1))
                    dd = smp.tile([128, 1], F32, name=f"{P}dd_{qt}", tag="ln_dd")
                    nc.vector.tensor_scalar_add(dd[:], sigma[:], LN_EPS)
                    rec2 = smp.tile([128, 1], F32, name=f"{P}rc_{qt}",
                                    tag="ln_rc")
                    nc.vector.reciprocal_approx_fast(rec2[:], dd[:])
                    eng_z = nc.gpsimd if qt % 2 == 0 else nc.vector
                    eng_z.tensor_scalar(o_t[:], yt[:], mv[:, 0:1], rec2[:],
                                        ALU.subtract, ALU.mult)
                    eng_a = nc.vector if qt % 2 == 0 else nc.gpsimd
                    eng_a.tensor_mul(o_t[:], o_t[:], lna_t[:])
                    nc.gpsimd.tensor_add(o_t[:], o_t[:], lnb_t[:])
                    nc.sync.dma_start(out_d[qt * 128:(qt + 1) * 128, :], o_t[:])

    nc.compile()
    return nc


def _get_nc():
    if "nc" not in _CACHE:
        _CACHE["nc"] = build()
    return _CACHE["nc"]


def _interleave8(a):
    """[D, N] f32 -> [128, 8, N] fp8 with [p, 2j+t, n] = a[j*256+t*128+p, n]."""
    n = a.shape[1]
    return np.ascontiguousarray(
        a.reshape(4, 2, 128, n).transpose(2, 0, 1, 3).reshape(128, 8, n)
    ).astype(NP8)


def _in_maps(q, w_qs, w_ks, w_vs, proj_w, proj_b, ln_a, ln_b):
    wq8 = _interleave8(np.ascontiguousarray(
        w_qs.transpose(1, 0, 2).reshape(D, H * DK)))
    wk8 = _interleave8(np.ascontiguousarray(
        w_ks.transpose(1, 0, 2).reshape(D, H * DK)))
    wv8 = _interleave8(np.ascontiguousarray(
        w_vs.transpose(1, 0, 2).reshape(D, H * DK)))
    pw8 = _interleave8(np.ascontiguousarray(proj_w.T))
    ones8 = np.ones((1, 2), NP8)
    lna = np.ascontiguousarray(ln_a[None, :]).astype(np.float32)
    lnb = np.ascontiguousarray(ln_b[None, :]).astype(np.float32)
    maps = []
    for c in range(8):
        b, half = c // 2, c % 2
        qb = q[b]
        perm = np.r_[half * HALF:(half + 1) * HALF,
                     (1 - half) * HALF:(2 - half) * HALF]
        qt8 = _interleave8(np.ascontiguousarray(qb.T[:, perm]))
        qres = np.ascontiguousarray(
            qb[half * HALF:(half + 1) * HALF, :] + proj_b[None, :]
        ).astype(np.float32)
        maps.append({
            "qt8": qt8, "qres": qres,
            "wq8": wq8, "wk8": wk8, "wv8": wv8, "pw8": pw8,
            "ones8": ones8, "lna": lna, "lnb": lnb,
        })
    return maps


def kernel(q, w_qs, w_ks, w_vs, proj_w, proj_b, ln_a, ln_b, **kw):
    q = np.asarray(q, dtype=np.float32)
    w_qs = np.asarray(w_qs, dtype=np.float32)
    w_ks = np.asarray(w_ks, dtype=np.float32)
    w_vs = np.asarray(w_vs, dtype=np.float32)
    proj_w = np.asarray(proj_w, dtype=np.float32)
    proj_b = np.asarray(proj_b, dtype=np.float32)
    ln_a = np.asarray(ln_a, dtype=np.float32)
    ln_b = np.asarray(ln_b, dtype=np.float32)

    in_maps = _in_maps(q, w_qs, w_ks, w_vs, proj_w, proj_b, ln_a, ln_b)
    nc = _get_nc()
    res = run_bass_kernel_spmd(nc, in_maps, core_ids=list(range(8))).results

    out = np.empty((B, L, D), dtype=np.float32)
    for c in range(8):
        b, half = c // 2, c % 2
        out[b, half * HALF:(half + 1) * HALF, :] = res[c]["out"]
    return out
